# revision 80
# baseline (speedup 1.0000x reference)
"""Fused multi-head attention kernel for Trainium2, 8-core SPMD.

Problem: B=4, S=2048, D=1024, H=16 heads of 64. y = attn(x) with torch-Linear
style projections (y = x @ W.T + b).

Sharding: core c -> (batch b = c//2, head-group g = c%2 covering 8 heads =
feature rows [512g, 512g+512) of wq/wk/wv and columns [512g, 512g+512) of wo).
Each core computes its heads' full SxS attention and a partial output
projection; the host sums the two partials per batch and adds wo_b.

v2 schedule (ACT-paced, PE kept gapless for the DVFS p-state ramp;
731us baseline -> ~421us):
  - all inputs bf16 (halves prologue DMA; PE rate is 1 cycle/col either
    way), host-packed so every weight DMA is one contiguous 2-D transfer.
  - logits in [j, i] orientation with two heads row-packed on the PE
    (tile_position (0,0)/(64,0)) - the packed pair streams concurrently.
  - exp on ACT as one [128, 1024] instruction per j-tile (widest the PSUM
    budget allows; ACT overhead ~0.3us/instr makes narrow exps lose).
    q/k bias-adds also ride the ACT (activation Identity with [P,1] bias)
    to keep the DVE queue clear for the i-block tails.
  - softmax denominator from a ones column interleaved into v (AV matmul
    M=65 puts it at psum row 64).  At each i-block tail two cheap copies
    evacuate the denominators + values out of PSUM (they gate the next
    i-block's AV accumulation via the psPre WAR), then the bf16 DVE
    reciprocals run off-chain into partition 0, and the deferred norm
    (flushed at jt==8 of the next block) is two Pool partition_broadcasts
    + two DVE multiplies - nothing on the PE.
    (Notes for posterity: reciprocal_approx_fast computes garbage in this
    toolchain; partition_broadcast only broadcasts the tile's partition 0;
    gpsimd cannot touch PSUM; ACT-side 1/d via exp(-ln d) forces
    1.5us act-table swaps; single-partition APs must start at 0 or 64.)
  - projection / output-projection matmuls are sliced into single-matmul
    work items popped 2-3 per j-tile between the exp and AV emissions, so
    the PE has independent work while exp(jt) is in flight; output
    projection runs with a 1-i-block lag right behind the deferred norms.
  - PSUM: lt pool 2x[128,1024] (4 banks) + pre pool 2x[128,512] (2 banks)
    + misc pool 2x[128,512] (2 banks) = exactly 8 banks.  The prologue
    borrows lt+pre for the k-major pair-0 q/k projection so the PE can
    start as soon as the first x chunk lands.
  - every DMA writes a freshly-allocated SBUF slot exactly once (single
    semaphore wait per DMA descriptor toolchain restriction).
"""

import numpy as np

B, S, D, HEAD_DIM = 4, 2048, 1024, 64
NHEADS = D // HEAD_DIM
N_CORES = 8
F = D // 2          # local features per core (8 heads * 64)
P = 128
NPAIR = 4           # head pairs per core
KT = D // P         # 8 contraction tiles for projections
NIB = 4             # i blocks of 512
IB = 512
NJT = S // P        # 16 j tiles
PAIRW = 2 * (HEAD_DIM + 1)  # [v_h0|ones|v_h1|ones] = 130 cols per pair
VW = NPAIR * PAIRW          # 520


def _build_program(repeat=1):
    import concourse.bass as bass
    import concourse.bacc as bacc
    import concourse.mybir as mybir
    import concourse.tile as tile

    f32 = mybir.dt.float32
    f32r = mybir.dt.float32r
    bf16 = mybir.dt.bfloat16
    Exp = mybir.ActivationFunctionType.Exp
    Identity = mybir.ActivationFunctionType.Identity

    nc = bacc.Bacc("TRN2", target_bir_lowering=False, debug=False, num_devices=N_CORES)

    # wq/wk host-packed as [m][P, KT*P] and wv as [P, KT*F] so each weight
    # DMA is one contiguous 2-D transfer (multi-descriptor rearranged DMAs
    # had slow triggers that held up the prologue).
    xT = nc.declare_dram_parameter("xT", [D, S], bf16, isOutput=False)
    wqP = nc.declare_dram_parameter("wqP", [NPAIR, P, KT * P], bf16, isOutput=False)
    wkP = nc.declare_dram_parameter("wkP", [NPAIR, P, KT * P], bf16, isOutput=False)
    wvP = nc.declare_dram_parameter("wvP", [P, KT * F], bf16, isOutput=False)
    woT = nc.declare_dram_parameter("woT", [F, D], bf16, isOutput=False)
    bq = nc.declare_dram_parameter("bq", [F], f32, isOutput=False)
    bk = nc.declare_dram_parameter("bk", [F], f32, isOutput=False)
    bv = nc.declare_dram_parameter("bv", [F], bf16, isOutput=False)
    ones = nc.declare_dram_parameter("ones", [P, P], f32r, isOutput=False)
    y = nc.declare_dram_parameter("y", [S, D], f32, isOutput=True)

    with tile.TileContext(nc) as tc:
        with (
            nc.allow_low_precision(reason="bf16 operands by design"),
            tc.tile_pool(name="pbias", bufs=1) as pbias,
            tc.tile_pool(name="px", bufs=8) as px,          # x chunks + wv
            tc.tile_pool(name="pw", bufs=4) as pw,          # wq/wk/wo weights
            tc.tile_pool(name="pqk", bufs=4) as pqk,        # q/k feature-major
            tc.tile_pool(name="ppre", bufs=4) as ppre,      # preout per pair
            tc.tile_pool(name="pv", bufs=16) as pv,         # v seq-major
            tc.tile_pool(name="pel", bufs=3) as pel,        # exp tiles
            tc.tile_pool(name="pps", bufs=4) as pps,        # pre_s cast tiles
            tc.tile_pool(name="prb", bufs=6) as prb,        # col/rsb/lnt/bc/osb
            tc.tile_pool(name="psLt", bufs=2, space="PSUM") as psLt,    # 4 banks
            tc.tile_pool(name="psPre", bufs=2, space="PSUM") as psPre,  # 2 banks
            tc.tile_pool(name="psMisc", bufs=2, space="PSUM") as psMisc,  # 2 banks
        ):
            # ---- one-time DMA loads (all into fresh slots) --------------
            # pair-0 q/k weights first so the prologue projection can start
            # as soon as x chunks land.
            wq_t, wk_t = [None] * NPAIR, [None] * NPAIR
            for m in (0,):
                wq_t[m] = pw.tile([P, KT, P], bf16, tag="wq", name=f"wq{m}")
                nc.sync.dma_start(
                    wq_t[m][:], wqP[m].rearrange("p (ko f) -> p ko f", ko=KT)
                )
                wk_t[m] = pw.tile([P, KT, P], bf16, tag="wk", name=f"wk{m}")
                nc.sync.dma_start(
                    wk_t[m][:], wkP[m].rearrange("p (ko f) -> p ko f", ko=KT)
                )
            xt = []
            for k in range(KT):
                t = px.tile([P, S], bf16, tag="x", name=f"xt{k}")
                nc.sync.dma_start(t[:], xT[k * P : (k + 1) * P, :])
                xt.append(t)
            bq_sb = pbias.tile([P, NPAIR], f32, tag="bq")
            bk_sb = pbias.tile([P, NPAIR], f32, tag="bk")
            nc.sync.dma_start(bq_sb[:], bq.rearrange("(o p) -> p o", p=P))
            nc.sync.dma_start(bk_sb[:], bk.rearrange("(o p) -> p o", p=P))
            bv_sb = pbias.tile([P, F], bf16, tag="bv")
            nc.sync.dma_start(bv_sb[:], bv[None, :].to_broadcast((P, F)))
            ones_sb = pbias.tile([P, P], f32r, tag="ones")
            nc.sync.dma_start(ones_sb[:], ones[:])
            for m in range(1, NPAIR):
                wq_t[m] = pw.tile([P, KT, P], bf16, tag="wq", name=f"wq{m}")
                nc.sync.dma_start(
                    wq_t[m][:], wqP[m].rearrange("p (ko f) -> p ko f", ko=KT)
                )
                wk_t[m] = pw.tile([P, KT, P], bf16, tag="wk", name=f"wk{m}")
                nc.sync.dma_start(
                    wk_t[m][:], wkP[m].rearrange("p (ko f) -> p ko f", ko=KT)
                )
            wv_all = px.tile([P, KT, F], bf16, tag="wv", name="wv_all")
            nc.sync.dma_start(
                wv_all[:], wvP.rearrange("p (ko f) -> p ko f", ko=KT)
            )
            wv_t = [wv_all[:, k, :] for k in range(KT)]
            wo_t = []
            for m in range(NPAIR):
                t = pw.tile([P, D], bf16, tag="wo", name=f"wo{m}")
                nc.sync.dma_start(t[:], woT[m * P : (m + 1) * P, :])
                wo_t.append(t)

            for _rep in range(repeat):
              R = f"{_rep}_"
              # q/k tiles per pair, created lazily (2 pairs in flight).
              qk_tiles = {}

              def get_qk(m):
                  if m not in qk_tiles:
                      qk_tiles[m] = (
                          pqk.tile([P, S], bf16, tag="qk", name=f"{R}q{m}"),
                          pqk.tile([P, S], bf16, tag="qk", name=f"{R}k{m}"),
                      )
                  return qk_tiles[m]

              def emit_bias_add(m, ns, which, ps):
                  # On the ACT engine (not DVE): keeps the DVE queue clear
                  # for the i-block-tail evacuation copies + reciprocals.
                  dst = get_qk(m)[which]
                  b_sb = bq_sb if which == 0 else bk_sb
                  nc.scalar.activation(
                      dst[:, ns * IB : (ns + 1) * IB],
                      ps,
                      Identity,
                      bias=b_sb[:, m : m + 1],
                  )

              # ---- prologue: pair-0 q/k projection, k-major, overlapping
              # the x DMAs.  Borrows lt/pre/misc psum (all idle here).
              get_qk(0)
              plt0 = psLt.tile([P, 2 * IB], f32, tag="lt", name=f"{R}plt0")
              plt1 = psLt.tile([P, 2 * IB], f32, tag="lt", name=f"{R}plt1")
              ppr0 = psPre.tile([P, IB], f32, tag="pre", name=f"{R}ppr0")
              ppr1 = psPre.tile([P, IB], f32, tag="pre", name=f"{R}ppr1")
              pms0 = psMisc.tile([P, IB], f32, tag="misc", name=f"{R}pms0")
              pms1 = psMisc.tile([P, IB], f32, tag="misc", name=f"{R}pms1")
              # (which, ns) -> psum slice
              pro_ps = {
                  (1, 0): plt0[:, 0:IB], (1, 1): plt0[:, IB : 2 * IB],
                  (1, 2): plt1[:, 0:IB], (1, 3): plt1[:, IB : 2 * IB],
                  (0, 0): ppr0[:], (0, 1): ppr1[:],
                  (0, 2): pms0[:], (0, 3): pms1[:],
              }
              pro_sets = list(pro_ps.keys())
              for k in range(KT):
                  for which, ns in pro_sets:
                      w_t = wq_t[0] if which == 0 else wk_t[0]
                      nc.tensor.matmul(
                          pro_ps[(which, ns)],
                          lhsT=w_t[:, k, :],
                          rhs=xt[k][:, ns * IB : (ns + 1) * IB],
                          start=(k == 0),
                          stop=(k == KT - 1),
                      )
              for which, ns in pro_sets:
                  emit_bias_add(0, ns, which, pro_ps[(which, ns)])

              # ---- v tiles with interleaved ones columns ------------------
              v_sb = []
              for jt in range(NJT):
                  t = pv.tile([P, VW], bf16, tag="v", name=f"{R}v{jt}")
                  vview = t[:].rearrange("p (m h c) -> p m h c", h=2, c=HEAD_DIM + 1)
                  nc.vector.tensor_copy(
                      vview[:, :, :, HEAD_DIM : HEAD_DIM + 1],
                      ones_sb[:, 0 : 2 * NPAIR].rearrange(
                          "p (m h) -> p m h", h=2
                      )[:, :, :, None],
                  )
                  v_sb.append(t)

              def emit_vproj(si):
                  ps = psMisc.tile([P, F], f32, tag="misc", name=f"{R}vps{si}")
                  for k in range(KT):
                      nc.tensor.matmul(
                          ps[:],
                          lhsT=xt[k][:, si * P : (si + 1) * P],
                          rhs=wv_t[k],
                          start=(k == 0),
                          stop=(k == KT - 1),
                      )
                  ps4 = ps[:].rearrange("p (m h c) -> p m h c", m=NPAIR, h=2)
                  bv4 = bv_sb[:].rearrange("p (m h c) -> p m h c", m=NPAIR, h=2)
                  vview = v_sb[si][:].rearrange(
                      "p (m h c) -> p m h c", h=2, c=HEAD_DIM + 1
                  )
                  nc.vector.tensor_add(
                      out=vview[:, :, :, 0:HEAD_DIM], in0=ps4, in1=bv4
                  )

              # ---- work queue: single-matmul items popped inside j-loops --
              work = []

              def enqueue_half(m, ns, which):
                  # 8 chunk-items per half; the psum slot is acquired by
                  # chunk 0 and released by the bias-add after chunk 7.
                  get_qk(m)
                  st = {}

                  def chunk(k, m=m, ns=ns, which=which, st=st):
                      if k == 0:
                          st["ps"] = psMisc.tile(
                              [P, IB], f32, tag="misc",
                              name=f"{R}pj{m}_{ns}_{which}",
                          )
                      w_t = wq_t[m] if which == 0 else wk_t[m]
                      nc.tensor.matmul(
                          st["ps"][:],
                          lhsT=w_t[:, k, :],
                          rhs=xt[k][:, ns * IB : (ns + 1) * IB],
                          start=(k == 0),
                          stop=(k == KT - 1),
                      )
                      if k == KT - 1:
                          emit_bias_add(m, ns, which, st["ps"][:])

                  for k in range(KT):
                      work.append(lambda k=k, chunk=chunk: chunk(k))

              def enqueue_proj(m):
                  for ns in range(NIB):
                      for which in (0, 1):
                          enqueue_half(m, ns, which)



              preout = []

              def enqueue_outproj(it):
                  for nb in range(2):
                      st = {}

                      def chunk(ft, it=it, nb=nb, st=st):
                          if ft == 0:
                              st["ps"] = psMisc.tile(
                                  [P, IB], f32, tag="misc",
                                  name=f"{R}ops{it}_{nb}",
                              )
                          nc.tensor.matmul(
                              st["ps"][:],
                              lhsT=preout[ft][:, it * P : (it + 1) * P],
                              rhs=wo_t[ft][:, nb * IB : (nb + 1) * IB],
                              start=(ft == 0),
                              stop=(ft == NPAIR - 1),
                          )
                          if ft == NPAIR - 1:
                              osb = prb.tile(
                                  [P, IB], f32, tag="rb",
                                  name=f"{R}osb{it}_{nb}",
                              )
                              nc.vector.tensor_copy(osb[:], st["ps"][:])
                              if _rep == 0:
                                  nc.sync.dma_start(
                                      y[it * P : (it + 1) * P,
                                        nb * IB : (nb + 1) * IB],
                                      osb[:],
                                  )

                      for ft in range(NPAIR):
                          work.append(lambda ft=ft, chunk=chunk: chunk(ft))

              pending_norm = [None]

              def flush_norm():
                  if pending_norm[0] is not None:
                      pending_norm[0]()
                      pending_norm[0] = None

              # ---- attention ---------------------------------------------
              for m in range(NPAIR):
                  if m < NPAIR - 1:
                      enqueue_proj(m + 1)
                  q_m, k_m = get_qk(m)
                  pre_m = ppre.tile([P, S], bf16, tag="pre", name=f"{R}pre{m}")
                  preout.append(pre_m)
                  for ib in range(NIB):
                      isl = slice(ib * IB, (ib + 1) * IB)
                      pre0 = psPre.tile(
                          [P, IB], f32, tag="pre", name=f"{R}pre0_{m}_{ib}"
                      )
                      pre1 = psPre.tile(
                          [P, IB], f32, tag="pre", name=f"{R}pre1_{m}_{ib}"
                      )
                      for jt in range(NJT):
                          if m == 0 and ib == 0:
                              emit_vproj(jt)
                          jsl = slice(jt * P, (jt + 1) * P)
                          lt = psLt.tile(
                              [P, 2 * IB], f32, tag="lt",
                              name=f"{R}l{m}_{ib}_{jt}",
                          )
                          nc.tensor.matmul(
                              lt[:, 0:IB],
                              lhsT=k_m[0:64, jsl],
                              rhs=q_m[0:64, isl],
                              start=True,
                              stop=True,
                              tile_position=(0, 0),
                          )
                          nc.tensor.matmul(
                              lt[:, IB : 2 * IB],
                              lhsT=k_m[64:128, jsl],
                              rhs=q_m[64:128, isl],
                              start=True,
                              stop=True,
                              tile_position=(64, 0),
                          )
                          et = pel.tile(
                              [P, 2 * IB], bf16, tag="e",
                              name=f"{R}e{m}_{ib}_{jt}",
                          )
                          nc.scalar.activation(et[:], lt[:], Exp, scale=0.125)
                          if jt == 8:
                              flush_norm()
                              if m == NPAIR - 1 and ib >= 1:
                                  for it in range(4 * (ib - 1), 4 * ib):
                                      enqueue_outproj(it)
                          if m == 0 and ib == 0:
                              npop = 0
                          elif m == NPAIR - 1 and jt >= 8:
                              npop = 3
                          else:
                              npop = 2
                          for _ in range(npop):
                              if work:
                                  work.pop(0)()
                          last = jt == NJT - 1
                          if last:
                              col = prb.tile(
                                  [P, IB], bf16, tag="rb", name=f"{R}c{m}_{ib}"
                              )
                              pre_s = pps.tile(
                                  [P, 2 * IB], bf16, tag="ps",
                                  name=f"{R}ps{m}_{ib}",
                              )
                          nc.tensor.matmul(
                              pre0[0:65, :],
                              lhsT=v_sb[jt][:, m * PAIRW : m * PAIRW + HEAD_DIM + 1],
                              rhs=et[:, 0:IB],
                              start=(jt == 0),
                              stop=last,
                          )
                          if last:
                              # evacuate pre0 right away: these copies gate
                              # the next i-block's AV writes (psPre WAR).
                              # The wide cast runs on the idle Pool engine so
                              # it overlaps the DVE denominator copy and does
                              # not queue ahead of the reciprocals.
                              nc.vector.tensor_copy(col[0:1, :], pre0[64:65, :])
                              nc.vector.tensor_copy(
                                  pre_s[0:64, 0:IB], pre0[0:64, :]
                              )
                          nc.tensor.matmul(
                              pre1[0:65, :],
                              lhsT=v_sb[jt][
                                  :, m * PAIRW + HEAD_DIM + 1 : (m + 1) * PAIRW
                              ],
                              rhs=et[:, IB : 2 * IB],
                              start=(jt == 0),
                              stop=last,
                          )
                          if last:
                              nc.vector.tensor_copy(col[64:65, :], pre1[64:65, :])
                              nc.vector.tensor_copy(
                                  pre_s[0:64, IB : 2 * IB], pre1[0:64, :]
                              )
                      # ---- i-block tail: reciprocals off the psum WAR chain.
                      # bf16 in/out makes the DVE 2x mode eligible; h1 first
                      # because only it gates the PE-side bc1 matmul at the
                      # deferred-norm flush.
                      # one partition-parallel reciprocal covers both
                      # denominator rows (0 and 64; rows 1-63 are unread
                      # garbage); the remap copy gives the h1 row a
                      # partition-0 home for partition_broadcast.
                      rsb = prb.tile([P, IB], bf16, tag="rb", name=f"{R}r{m}_{ib}")
                      rsb2 = prb.tile([P, IB], bf16, tag="rb", name=f"{R}r2{m}_{ib}")
                      nc.vector.reciprocal(rsb[0:65, :], col[0:65, :])
                      nc.vector.tensor_copy(rsb2[0:1, :], rsb[64:65, :])

                      def norm(m=m, ib=ib, isl=isl, rsb=rsb, rsb2=rsb2,
                               pre_s=pre_s, pre_m=pre_m):
                          bc0 = prb.tile(
                              [P, IB], bf16, tag="rb", name=f"{R}bc0_{m}_{ib}"
                          )
                          nc.gpsimd.partition_broadcast(
                              bc0[:], rsb[0:1, :], channels=P
                          )
                          bc1 = prb.tile(
                              [P, IB], bf16, tag="rb", name=f"{R}bc1_{m}_{ib}"
                          )
                          nc.gpsimd.partition_broadcast(
                              bc1[:], rsb2[0:1, :], channels=P
                          )
                          nc.vector.tensor_mul(
                              out=pre_m[0:64, isl],
                              in0=pre_s[0:64, 0:IB],
                              in1=bc0[0:64, :],
                          )
                          nc.vector.tensor_mul(
                              out=pre_m[64:128, isl],
                              in0=pre_s[0:64, IB : 2 * IB],
                              in1=bc1[0:64, :],
                          )

                      pending_norm[0] = norm

              while work:
                  work.pop(0)()
              flush_norm()
              for it in range(12, S // P):
                  enqueue_outproj(it)
              while work:
                  work.pop(0)()

    nc.compile()
    return nc


_NC = None


def _get_program():
    global _NC
    if _NC is None:
        _NC = _build_program()
    return _NC


def make_in_maps(x, wq_w, wq_b, wk_w, wk_b, wv_w, wv_b, wo_w, wo_b):
    import ml_dtypes

    bf = ml_dtypes.bfloat16
    x = np.asarray(x, dtype=np.float32)
    in_maps = []
    wqT_f = np.ascontiguousarray(np.asarray(wq_w, dtype=np.float32).T)  # [D, D]
    wkT_f = np.ascontiguousarray(np.asarray(wk_w, dtype=np.float32).T)
    wvT_f = np.ascontiguousarray(np.asarray(wv_w, dtype=np.float32).T)
    woT_f = np.ascontiguousarray(np.asarray(wo_w, dtype=np.float32).T)  # [D, D]

    def pack_kmajor(wT_local, ncols):
        # [D, ncols*NPAIR?] block -> [P, KT*ncols] with k-chunks contiguous
        return np.ascontiguousarray(
            wT_local.reshape(KT, P, ncols).transpose(1, 0, 2).reshape(P, -1)
        )
    ones = np.ones((P, P), dtype=np.float32)
    for c in range(N_CORES):
        b, g = divmod(c, 2)
        fs = slice(g * F, (g + 1) * F)
        in_maps.append(
            {
                "xT": np.ascontiguousarray(x[b].T.astype(bf)),
                "wqP": np.stack(
                    [
                        pack_kmajor(
                            wqT_f[:, fs][:, m * P : (m + 1) * P].astype(bf), P
                        )
                        for m in range(NPAIR)
                    ]
                ),
                "wkP": np.stack(
                    [
                        pack_kmajor(
                            wkT_f[:, fs][:, m * P : (m + 1) * P].astype(bf), P
                        )
                        for m in range(NPAIR)
                    ]
                ),
                "wvP": pack_kmajor(wvT_f[:, fs].astype(bf), F),
                "woT": np.ascontiguousarray(woT_f[fs, :].astype(bf)),
                "bq": np.ascontiguousarray(np.asarray(wq_b, np.float32)[fs]),
                "bk": np.ascontiguousarray(np.asarray(wk_b, np.float32)[fs]),
                "bv": np.ascontiguousarray(
                    np.asarray(wv_b, np.float32)[fs].astype(bf)
                ),
                "ones": ones,
            }
        )
    return in_maps


def gather_output(results, wo_b):
    wo_b = np.asarray(wo_b, dtype=np.float32)
    out = np.empty((B, S, D), dtype=np.float32)
    for b in range(B):
        out[b] = results[2 * b]["y"] + results[2 * b + 1]["y"] + wo_b
    return out


def kernel(x, wq_w, wq_b, wk_w, wk_b, wv_w, wv_b, wo_w, wo_b):
    from concourse.bass_utils import run_bass_kernel_spmd

    nc = _get_program()
    in_maps = make_in_maps(x, wq_w, wq_b, wk_w, wk_b, wv_w, wv_b, wo_w, wo_b)
    res = run_bass_kernel_spmd(nc, in_maps, list(range(N_CORES)))
    return gather_output(res.results, wo_b)


# revision 81
# speedup vs baseline: 1.1298x; 1.1298x over previous
"""Fused multi-head attention kernel for Trainium2, 8-core SPMD.

Problem: B=4, S=2048, D=1024, H=16 heads of 64. y = attn(x) with torch-Linear
style projections (y = x @ W.T + b).

Sharding: core c -> (batch b = c//2, head-group g = c%2 covering 8 heads =
feature rows [512g, 512g+512) of wq/wk/wv and columns [512g, 512g+512) of wo).
Each core computes its heads' full SxS attention and a partial output
projection; the host sums the two partials per batch and adds wo_b.

v2 schedule (ACT-paced, PE kept gapless for the DVFS p-state ramp;
731us baseline -> ~421us):
  - all inputs bf16 (halves prologue DMA; PE rate is 1 cycle/col either
    way), host-packed so every weight DMA is one contiguous 2-D transfer.
  - logits in [j, i] orientation with two heads row-packed on the PE
    (tile_position (0,0)/(64,0)) - the packed pair streams concurrently.
  - exp on ACT as one [128, 1024] instruction per j-tile (widest the PSUM
    budget allows; ACT overhead ~0.3us/instr makes narrow exps lose).
    q/k bias-adds also ride the ACT (activation Identity with [P,1] bias)
    to keep the DVE queue clear for the i-block tails.
  - softmax denominator from a ones column interleaved into v (AV matmul
    M=65 puts it at psum row 64).  At each i-block tail two cheap copies
    evacuate the denominators + values out of PSUM (they gate the next
    i-block's AV accumulation via the psPre WAR), then the bf16 DVE
    reciprocals run off-chain into partition 0, and the deferred norm
    (flushed at jt==8 of the next block) is two Pool partition_broadcasts
    + two DVE multiplies - nothing on the PE.
    (Notes for posterity: reciprocal_approx_fast computes garbage in this
    toolchain; partition_broadcast only broadcasts the tile's partition 0;
    gpsimd cannot touch PSUM; ACT-side 1/d via exp(-ln d) forces
    1.5us act-table swaps; single-partition APs must start at 0 or 64.)
  - projection / output-projection matmuls are sliced into single-matmul
    work items popped 2-3 per j-tile between the exp and AV emissions, so
    the PE has independent work while exp(jt) is in flight; output
    projection runs with a 1-i-block lag right behind the deferred norms.
  - PSUM: lt pool 2x[128,1024] (4 banks) + pre pool 2x[128,512] (2 banks)
    + misc pool 2x[128,512] (2 banks) = exactly 8 banks.  The prologue
    borrows lt+pre for the k-major pair-0 q/k projection so the PE can
    start as soon as the first x chunk lands.
  - every DMA writes a freshly-allocated SBUF slot exactly once (single
    semaphore wait per DMA descriptor toolchain restriction).
"""

import numpy as np

B, S, D, HEAD_DIM = 4, 2048, 1024, 64
NHEADS = D // HEAD_DIM
N_CORES = 8
F = D // 2          # local features per core (8 heads * 64)
P = 128
NPAIR = 4           # head pairs per core
KT = D // P         # 8 contraction tiles for projections
NIB = 4             # i blocks of 512
IB = 512
NJT = S // P        # 16 j tiles
PAIRW = 2 * (HEAD_DIM + 1)  # [v_h0|ones|v_h1|ones] = 130 cols per pair
VW = NPAIR * PAIRW          # 520


def _build_program(repeat=1):
    import concourse.bass as bass
    import concourse.bacc as bacc
    import concourse.mybir as mybir
    import concourse.tile as tile

    f32 = mybir.dt.float32
    f32r = mybir.dt.float32r
    bf16 = mybir.dt.bfloat16
    Exp = mybir.ActivationFunctionType.Exp
    Identity = mybir.ActivationFunctionType.Identity

    nc = bacc.Bacc("TRN2", target_bir_lowering=False, debug=False, num_devices=N_CORES)

    # wq/wk host-packed as [m][P, KT*P] and wv as [P, KT*F] so each weight
    # DMA is one contiguous 2-D transfer (multi-descriptor rearranged DMAs
    # had slow triggers that held up the prologue).
    xT = nc.declare_dram_parameter("xT", [D, S], bf16, isOutput=False)
    wqP = nc.declare_dram_parameter("wqP", [NPAIR, P, KT * P], bf16, isOutput=False)
    wkP = nc.declare_dram_parameter("wkP", [NPAIR, P, KT * P], bf16, isOutput=False)
    wvP = nc.declare_dram_parameter("wvP", [P, KT * F], bf16, isOutput=False)
    woT = nc.declare_dram_parameter("woT", [F, D], bf16, isOutput=False)
    bq = nc.declare_dram_parameter("bq", [F], f32, isOutput=False)
    bk = nc.declare_dram_parameter("bk", [F], f32, isOutput=False)
    bv = nc.declare_dram_parameter("bv", [F], bf16, isOutput=False)
    ones = nc.declare_dram_parameter("ones", [P, P], f32r, isOutput=False)
    y = nc.declare_dram_parameter("y", [S, D], f32, isOutput=True)

    with tile.TileContext(nc) as tc:
        with (
            nc.allow_low_precision(reason="bf16 operands by design"),
            tc.tile_pool(name="pbias", bufs=1) as pbias,
            tc.tile_pool(name="px", bufs=8) as px,          # x chunks + wv
            tc.tile_pool(name="pw", bufs=4) as pw,          # wq/wk/wo weights
            tc.tile_pool(name="pqk", bufs=4) as pqk,        # q/k feature-major
            tc.tile_pool(name="ppre", bufs=4) as ppre,      # preout per pair
            tc.tile_pool(name="pv", bufs=16) as pv,         # v seq-major
            tc.tile_pool(name="pel", bufs=3) as pel,        # exp tiles
            tc.tile_pool(name="pps", bufs=4) as pps,        # pre_s cast tiles
            tc.tile_pool(name="prb", bufs=6) as prb,        # col/rsb/lnt/bc/osb
            tc.tile_pool(name="psLt", bufs=2, space="PSUM") as psLt,    # 4 banks
            tc.tile_pool(name="psPre", bufs=2, space="PSUM") as psPre,  # 2 banks
            tc.tile_pool(name="psMisc", bufs=2, space="PSUM") as psMisc,  # 2 banks
        ):
            # ---- one-time DMA loads (all into fresh slots) --------------
            # pair-0 q/k weights first so the prologue projection can start
            # as soon as x chunks land.
            wq_t, wk_t = [None] * NPAIR, [None] * NPAIR
            for m in (0,):
                wq_t[m] = pw.tile([P, KT, P], bf16, tag="wq", name=f"wq{m}")
                nc.sync.dma_start(
                    wq_t[m][:], wqP[m].rearrange("p (ko f) -> p ko f", ko=KT)
                )
                wk_t[m] = pw.tile([P, KT, P], bf16, tag="wk", name=f"wk{m}")
                nc.sync.dma_start(
                    wk_t[m][:], wkP[m].rearrange("p (ko f) -> p ko f", ko=KT)
                )
            xt = []
            for k in range(KT):
                t = px.tile([P, S], bf16, tag="x", name=f"xt{k}")
                nc.sync.dma_start(t[:], xT[k * P : (k + 1) * P, :])
                xt.append(t)
            bq_sb = pbias.tile([P, NPAIR], f32, tag="bq")
            bk_sb = pbias.tile([P, NPAIR], f32, tag="bk")
            nc.sync.dma_start(bq_sb[:], bq.rearrange("(o p) -> p o", p=P))
            nc.sync.dma_start(bk_sb[:], bk.rearrange("(o p) -> p o", p=P))
            bv_sb = pbias.tile([P, F], bf16, tag="bv")
            nc.sync.dma_start(bv_sb[:], bv[None, :].to_broadcast((P, F)))
            ones_sb = pbias.tile([P, P], f32r, tag="ones")
            nc.sync.dma_start(ones_sb[:], ones[:])
            for m in range(1, NPAIR):
                wq_t[m] = pw.tile([P, KT, P], bf16, tag="wq", name=f"wq{m}")
                nc.sync.dma_start(
                    wq_t[m][:], wqP[m].rearrange("p (ko f) -> p ko f", ko=KT)
                )
                wk_t[m] = pw.tile([P, KT, P], bf16, tag="wk", name=f"wk{m}")
                nc.sync.dma_start(
                    wk_t[m][:], wkP[m].rearrange("p (ko f) -> p ko f", ko=KT)
                )
            wv_all = px.tile([P, KT, F], bf16, tag="wv", name="wv_all")
            nc.sync.dma_start(
                wv_all[:], wvP.rearrange("p (ko f) -> p ko f", ko=KT)
            )
            wv_t = [wv_all[:, k, :] for k in range(KT)]
            wo_t = []
            for m in range(NPAIR):
                t = pw.tile([P, D], bf16, tag="wo", name=f"wo{m}")
                nc.sync.dma_start(t[:], woT[m * P : (m + 1) * P, :])
                wo_t.append(t)

            for _rep in range(repeat):
              R = f"{_rep}_"
              # q/k tiles per pair, created lazily (2 pairs in flight).
              qk_tiles = {}

              def get_qk(m):
                  if m not in qk_tiles:
                      qk_tiles[m] = (
                          pqk.tile([P, S], bf16, tag="qk", name=f"{R}q{m}"),
                          pqk.tile([P, S], bf16, tag="qk", name=f"{R}k{m}"),
                      )
                  return qk_tiles[m]

              def emit_bias_add(m, ns, which, ps):
                  # On the ACT engine (not DVE): keeps the DVE queue clear
                  # for the i-block-tail evacuation copies + reciprocals.
                  dst = get_qk(m)[which]
                  b_sb = bq_sb if which == 0 else bk_sb
                  nc.scalar.activation(
                      dst[:, ns * IB : (ns + 1) * IB],
                      ps,
                      Identity,
                      bias=b_sb[:, m : m + 1],
                  )

              # ---- prologue: pair-0 q/k projection, k-major, overlapping
              # the x DMAs.  Borrows lt/pre/misc psum (all idle here).
              get_qk(0)
              plt0 = psLt.tile([P, 2 * IB], f32, tag="lt", name=f"{R}plt0")
              plt1 = psLt.tile([P, 2 * IB], f32, tag="lt", name=f"{R}plt1")
              ppr0 = psPre.tile([P, IB], f32, tag="pre", name=f"{R}ppr0")
              ppr1 = psPre.tile([P, IB], f32, tag="pre", name=f"{R}ppr1")
              pms0 = psMisc.tile([P, IB], f32, tag="misc", name=f"{R}pms0")
              pms1 = psMisc.tile([P, IB], f32, tag="misc", name=f"{R}pms1")
              # (which, ns) -> psum slice
              pro_ps = {
                  (1, 0): plt0[:, 0:IB], (1, 1): plt0[:, IB : 2 * IB],
                  (1, 2): plt1[:, 0:IB], (1, 3): plt1[:, IB : 2 * IB],
                  (0, 0): ppr0[:], (0, 1): ppr1[:],
                  (0, 2): pms0[:], (0, 3): pms1[:],
              }
              pro_sets = list(pro_ps.keys())
              for k in range(KT):
                  for which, ns in pro_sets:
                      w_t = wq_t[0] if which == 0 else wk_t[0]
                      nc.tensor.matmul(
                          pro_ps[(which, ns)],
                          lhsT=w_t[:, k, :],
                          rhs=xt[k][:, ns * IB : (ns + 1) * IB],
                          start=(k == 0),
                          stop=(k == KT - 1),
                      )
              for which, ns in pro_sets:
                  emit_bias_add(0, ns, which, pro_ps[(which, ns)])

              # ---- v tiles with interleaved ones columns ------------------
              v_sb = []
              for jt in range(NJT):
                  t = pv.tile([P, VW], bf16, tag="v", name=f"{R}v{jt}")
                  vview = t[:].rearrange("p (m h c) -> p m h c", h=2, c=HEAD_DIM + 1)
                  nc.vector.tensor_copy(
                      vview[:, :, :, HEAD_DIM : HEAD_DIM + 1],
                      ones_sb[:, 0 : 2 * NPAIR].rearrange(
                          "p (m h) -> p m h", h=2
                      )[:, :, :, None],
                  )
                  v_sb.append(t)

              def emit_vproj(si):
                  ps = psMisc.tile([P, F], f32, tag="misc", name=f"{R}vps{si}")
                  for k in range(KT):
                      nc.tensor.matmul(
                          ps[:],
                          lhsT=xt[k][:, si * P : (si + 1) * P],
                          rhs=wv_t[k],
                          start=(k == 0),
                          stop=(k == KT - 1),
                      )
                  ps4 = ps[:].rearrange("p (m h c) -> p m h c", m=NPAIR, h=2)
                  bv4 = bv_sb[:].rearrange("p (m h c) -> p m h c", m=NPAIR, h=2)
                  vview = v_sb[si][:].rearrange(
                      "p (m h c) -> p m h c", h=2, c=HEAD_DIM + 1
                  )
                  nc.vector.tensor_add(
                      out=vview[:, :, :, 0:HEAD_DIM], in0=ps4, in1=bv4
                  )

              # ---- work queue: single-matmul items popped inside j-loops --
              work = []

              def enqueue_half(m, ns, which):
                  # 8 chunk-items per half; the psum slot is acquired by
                  # chunk 0 and released by the bias-add after chunk 7.
                  get_qk(m)
                  st = {}

                  def chunk(k, m=m, ns=ns, which=which, st=st):
                      if k == 0:
                          st["ps"] = psMisc.tile(
                              [P, IB], f32, tag="misc",
                              name=f"{R}pj{m}_{ns}_{which}",
                          )
                      w_t = wq_t[m] if which == 0 else wk_t[m]
                      nc.tensor.matmul(
                          st["ps"][:],
                          lhsT=w_t[:, k, :],
                          rhs=xt[k][:, ns * IB : (ns + 1) * IB],
                          start=(k == 0),
                          stop=(k == KT - 1),
                      )
                      if k == KT - 1:
                          emit_bias_add(m, ns, which, st["ps"][:])

                  for k in range(KT):
                      work.append(lambda k=k, chunk=chunk: chunk(k))

              def enqueue_proj(m):
                  for ns in range(NIB):
                      for which in (0, 1):
                          enqueue_half(m, ns, which)



              preout = []

              def enqueue_outproj(it):
                  for nb in range(2):
                      st = {}

                      def chunk(ft, it=it, nb=nb, st=st):
                          if ft == 0:
                              st["ps"] = psMisc.tile(
                                  [P, IB], f32, tag="misc",
                                  name=f"{R}ops{it}_{nb}",
                              )
                          nc.tensor.matmul(
                              st["ps"][:],
                              lhsT=preout[ft][:, it * P : (it + 1) * P],
                              rhs=wo_t[ft][:, nb * IB : (nb + 1) * IB],
                              start=(ft == 0),
                              stop=(ft == NPAIR - 1),
                          )
                          if ft == NPAIR - 1:
                              osb = prb.tile(
                                  [P, IB], f32, tag="rb",
                                  name=f"{R}osb{it}_{nb}",
                              )
                              nc.vector.tensor_copy(osb[:], st["ps"][:])
                              if _rep == 0:
                                  nc.sync.dma_start(
                                      y[it * P : (it + 1) * P,
                                        nb * IB : (nb + 1) * IB],
                                      osb[:],
                                  )

                      for ft in range(NPAIR):
                          work.append(lambda ft=ft, chunk=chunk: chunk(ft))

              pending_norm = [None]

              def flush_norm():
                  if pending_norm[0] is not None:
                      pending_norm[0]()
                      pending_norm[0] = None

              # ---- attention ---------------------------------------------
              for m in range(NPAIR):
                  if m < NPAIR - 1:
                      enqueue_proj(m + 1)
                  q_m, k_m = get_qk(m)
                  pre_m = ppre.tile([P, S], bf16, tag="pre", name=f"{R}pre{m}")
                  preout.append(pre_m)
                  for ib in range(NIB):
                      isl = slice(ib * IB, (ib + 1) * IB)
                      pre0 = psPre.tile(
                          [P, IB], f32, tag="pre", name=f"{R}pre0_{m}_{ib}"
                      )
                      pre1 = psPre.tile(
                          [P, IB], f32, tag="pre", name=f"{R}pre1_{m}_{ib}"
                      )
                      for jt in range(NJT):
                          if m == 0 and ib == 0:
                              emit_vproj(jt)
                          jsl = slice(jt * P, (jt + 1) * P)
                          lt = psLt.tile(
                              [P, 2 * IB], f32, tag="lt",
                              name=f"{R}l{m}_{ib}_{jt}",
                          )
                          nc.tensor.matmul(
                              lt[:, 0:IB],
                              lhsT=k_m[0:64, jsl],
                              rhs=q_m[0:64, isl],
                              start=True,
                              stop=True,
                              tile_position=(0, 0),
                          )
                          nc.tensor.matmul(
                              lt[:, IB : 2 * IB],
                              lhsT=k_m[64:128, jsl],
                              rhs=q_m[64:128, isl],
                              start=True,
                              stop=True,
                              tile_position=(64, 0),
                          )
                          et = pel.tile(
                              [P, 2 * IB], bf16, tag="e",
                              name=f"{R}e{m}_{ib}_{jt}",
                          )
                          nc.scalar.activation(et[:], lt[:], Exp, scale=0.125)
                          if jt == 8:
                              flush_norm()
                              if m == NPAIR - 1 and ib >= 1:
                                  for it in range(4 * (ib - 1), 4 * ib):
                                      enqueue_outproj(it)
                          if m == 0 and ib == 0:
                              npop = 0
                          elif m == NPAIR - 1 and jt >= 8:
                              npop = 3
                          else:
                              npop = 2
                          for _ in range(npop):
                              if work:
                                  work.pop(0)()
                          last = jt == NJT - 1
                          if last:
                              col = prb.tile(
                                  [P, IB], bf16, tag="rb", name=f"{R}c{m}_{ib}"
                              )
                              pre_s = pps.tile(
                                  [P, 2 * IB], bf16, tag="ps",
                                  name=f"{R}ps{m}_{ib}",
                              )
                          nc.tensor.matmul(
                              pre0[0:65, :],
                              lhsT=v_sb[jt][:, m * PAIRW : m * PAIRW + HEAD_DIM + 1],
                              rhs=et[:, 0:IB],
                              start=(jt == 0),
                              stop=last,
                          )
                          if last:
                              # evacuate pre0 right away: these copies gate
                              # the next i-block's AV writes (psPre WAR).
                              # The wide cast runs on the idle Pool engine so
                              # it overlaps the DVE denominator copy and does
                              # not queue ahead of the reciprocals.
                              nc.vector.tensor_copy(col[0:1, :], pre0[64:65, :])
                              nc.vector.tensor_copy(
                                  pre_s[0:64, 0:IB], pre0[0:64, :]
                              )
                          nc.tensor.matmul(
                              pre1[0:65, :],
                              lhsT=v_sb[jt][
                                  :, m * PAIRW + HEAD_DIM + 1 : (m + 1) * PAIRW
                              ],
                              rhs=et[:, IB : 2 * IB],
                              start=(jt == 0),
                              stop=last,
                          )
                          if last:
                              nc.vector.tensor_copy(col[64:65, :], pre1[64:65, :])
                              nc.vector.tensor_copy(
                                  pre_s[0:64, IB : 2 * IB], pre1[0:64, :]
                              )
                      # ---- i-block tail: reciprocals off the psum WAR chain.
                      # bf16 in/out makes the DVE 2x mode eligible; h1 first
                      # because only it gates the PE-side bc1 matmul at the
                      # deferred-norm flush.
                      # two [1,512] reciprocals (NOT one [0:65,:] call: DVE
                      # reciprocal cost scales with total elements, not
                      # per-partition free size)
                      rsb = prb.tile([P, IB], bf16, tag="rb", name=f"{R}r{m}_{ib}")
                      rsb2 = prb.tile([P, IB], bf16, tag="rb", name=f"{R}r2{m}_{ib}")
                      nc.vector.reciprocal(rsb2[0:1, :], col[64:65, :])
                      nc.vector.reciprocal(rsb[0:1, :], col[0:1, :])

                      def norm(m=m, ib=ib, isl=isl, rsb=rsb, rsb2=rsb2,
                               pre_s=pre_s, pre_m=pre_m):
                          bc0 = prb.tile(
                              [P, IB], bf16, tag="rb", name=f"{R}bc0_{m}_{ib}"
                          )
                          nc.gpsimd.partition_broadcast(
                              bc0[:], rsb[0:1, :], channels=P
                          )
                          bc1 = prb.tile(
                              [P, IB], bf16, tag="rb", name=f"{R}bc1_{m}_{ib}"
                          )
                          nc.gpsimd.partition_broadcast(
                              bc1[:], rsb2[0:1, :], channels=P
                          )
                          nc.vector.tensor_mul(
                              out=pre_m[0:64, isl],
                              in0=pre_s[0:64, 0:IB],
                              in1=bc0[0:64, :],
                          )
                          nc.vector.tensor_mul(
                              out=pre_m[64:128, isl],
                              in0=pre_s[0:64, IB : 2 * IB],
                              in1=bc1[0:64, :],
                          )

                      pending_norm[0] = norm

              while work:
                  work.pop(0)()
              flush_norm()
              for it in range(12, S // P):
                  enqueue_outproj(it)
              while work:
                  work.pop(0)()

    nc.compile()
    return nc


_NC = None


def _get_program():
    global _NC
    if _NC is None:
        _NC = _build_program()
    return _NC


def make_in_maps(x, wq_w, wq_b, wk_w, wk_b, wv_w, wv_b, wo_w, wo_b):
    import ml_dtypes

    bf = ml_dtypes.bfloat16
    x = np.asarray(x, dtype=np.float32)
    in_maps = []
    wqT_f = np.ascontiguousarray(np.asarray(wq_w, dtype=np.float32).T)  # [D, D]
    wkT_f = np.ascontiguousarray(np.asarray(wk_w, dtype=np.float32).T)
    wvT_f = np.ascontiguousarray(np.asarray(wv_w, dtype=np.float32).T)
    woT_f = np.ascontiguousarray(np.asarray(wo_w, dtype=np.float32).T)  # [D, D]

    def pack_kmajor(wT_local, ncols):
        # [D, ncols*NPAIR?] block -> [P, KT*ncols] with k-chunks contiguous
        return np.ascontiguousarray(
            wT_local.reshape(KT, P, ncols).transpose(1, 0, 2).reshape(P, -1)
        )
    ones = np.ones((P, P), dtype=np.float32)
    for c in range(N_CORES):
        b, g = divmod(c, 2)
        fs = slice(g * F, (g + 1) * F)
        in_maps.append(
            {
                "xT": np.ascontiguousarray(x[b].T.astype(bf)),
                "wqP": np.stack(
                    [
                        pack_kmajor(
                            wqT_f[:, fs][:, m * P : (m + 1) * P].astype(bf), P
                        )
                        for m in range(NPAIR)
                    ]
                ),
                "wkP": np.stack(
                    [
                        pack_kmajor(
                            wkT_f[:, fs][:, m * P : (m + 1) * P].astype(bf), P
                        )
                        for m in range(NPAIR)
                    ]
                ),
                "wvP": pack_kmajor(wvT_f[:, fs].astype(bf), F),
                "woT": np.ascontiguousarray(woT_f[fs, :].astype(bf)),
                "bq": np.ascontiguousarray(np.asarray(wq_b, np.float32)[fs]),
                "bk": np.ascontiguousarray(np.asarray(wk_b, np.float32)[fs]),
                "bv": np.ascontiguousarray(
                    np.asarray(wv_b, np.float32)[fs].astype(bf)
                ),
                "ones": ones,
            }
        )
    return in_maps


def gather_output(results, wo_b):
    wo_b = np.asarray(wo_b, dtype=np.float32)
    out = np.empty((B, S, D), dtype=np.float32)
    for b in range(B):
        out[b] = results[2 * b]["y"] + results[2 * b + 1]["y"] + wo_b
    return out


def kernel(x, wq_w, wq_b, wk_w, wk_b, wv_w, wv_b, wo_w, wo_b):
    from concourse.bass_utils import run_bass_kernel_spmd

    nc = _get_program()
    in_maps = make_in_maps(x, wq_w, wq_b, wk_w, wk_b, wv_w, wv_b, wo_w, wo_b)
    res = run_bass_kernel_spmd(nc, in_maps, list(range(N_CORES)))
    return gather_output(res.results, wo_b)
